# revision 37
# baseline (speedup 1.0000x reference)
"""Trainium2 Bass kernel: 3-layer LSTM LM (embed -> 3xLSTM(H=256) -> FC 32000 -> log_softmax).

Strategy: data-parallel over batch across 8 cores (2 sequences per core).
Everything else (LSTM recurrence, FC, log_softmax over full vocab) is local
per core; zero collectives.
"""

import sys

sys.path.insert(0, "/opt/trn_rl_repo")

import numpy as np

import concourse.bass as bass
import concourse.mybir as mybir
import concourse.tile as tile
from concourse import bacc
from concourse.bass_utils import run_bass_kernel_spmd
from concourse.masks import make_identity
from concourse.tile import add_dep_helper

# Problem dims
V = 32000
E = 200
H = 256
B = 16
T = 256
N_CORES = 8
B_LOC = B // N_CORES  # 2 sequences per core
G4 = 4 * H  # 1024 gate width

# Tiling
CHUNK = 8  # recurrence steps per xg-precompute chunk (also the layer lag)
N_MCHUNK = G4 // 128  # 8 gate row chunks
VCHUNK = 512
SLOT = 512  # gate psum slot stride in fp32 elems == one 2KB bank
FP16 = mybir.dt.float16
FP32 = mybir.dt.float32
AF = mybir.ActivationFunctionType
LAYER_DIMS = [E, H, H]


def mkap(tile_ap, off, dims):
    """Custom strided AP on a tile: off in elements, dims=[[step,count],...]."""
    return bass.AP(tile_ap.tensor, off,
                   [list(tile_ap.ap[0])] + [list(d) for d in dims])


def ksizes(dim):
    """Partition-chunk sizes for a contraction dim."""
    out = []
    while dim > 0:
        out.append(min(dim, 128))
        dim -= 128
    return out


def build_nc(t_steps=T, has_bias=False):
    nsteps = t_steps
    nchunks = nsteps // CHUNK if nsteps >= CHUNK else 1
    chunk = min(CHUNK, nsteps)
    cb = chunk * B_LOC
    tb = nsteps * B_LOC
    ntok = nsteps * B_LOC
    n_gtiles = (ntok + 127) // 128

    nc = bacc.Bacc("TRN2", target_bir_lowering=False, debug=False,
                   num_devices=N_CORES)

    # DRAM I/O
    xids_d = nc.dram_tensor("xids", [ntok, 1], mybir.dt.int32, kind="ExternalInput")
    emb_d = nc.dram_tensor("emb", [V, E], FP32, kind="ExternalInput")
    wiT_d = [nc.dram_tensor(f"wiT{l}", [LAYER_DIMS[l], G4], FP16, kind="ExternalInput")
             for l in range(3)]
    whT_d = [nc.dram_tensor(f"whT{l}", [H, G4], FP16, kind="ExternalInput")
             for l in range(3)]
    bvec_d = [nc.dram_tensor(f"bvec{l}", [1, G4], FP16, kind="ExternalInput")
              for l in range(3)]
    fcWT_d = nc.dram_tensor("fcWT", [H, V], FP16, kind="ExternalInput")
    fcb_d = nc.dram_tensor("fcb", [1, V], FP16, kind="ExternalInput")
    # bf16 output (halves the output DMA); host casts back to fp32
    out_d = nc.dram_tensor("out", [tb, V], mybir.dt.bfloat16, kind="ExternalOutput")

    with tile.TileContext(nc, num_cores=N_CORES) as tc:
        with (
            tc.tile_pool(name="weights", bufs=1) as wpool,
            tc.tile_pool(name="state", bufs=1) as spool,
            tc.tile_pool(name="work", bufs=3) as work,
            tc.tile_pool(name="fcw", bufs=3) as fcwpool,
            tc.tile_pool(name="stage", bufs=3) as stpool,
        ):
            Bb = B_LOC
            lag = chunk
            nch = max(1, nsteps // chunk)

            # ---- Phase 0: allocate weight tiles (DMAs emitted after the
            # embedding gather so the gather-transpose chain starts first) ----
            wiT_sb = []
            whT_sb = []
            bvec_sb = []
            for l in range(3):
                ks = ksizes(LAYER_DIMS[l])
                wi = wpool.tile([128, len(ks) * G4], FP16, tag=f"wiT{l}", name=f"wiT{l}")
                wiT_sb.append(wi)
                wh = wpool.tile([128, 2 * G4], FP16, tag=f"whT{l}", name=f"whT{l}")
                whT_sb.append(wh)
                bv = wpool.tile([1, G4], FP16, tag=f"bvec{l}", name=f"bvec{l}")
                bvec_sb.append(bv)

            def emit_weight_dmas():
                for l in range(3):
                    ks = ksizes(LAYER_DIMS[l])
                    for kc, ksz in enumerate(ks):
                        nc.sync.dma_start(
                            wiT_sb[l][0:ksz, kc * G4:(kc + 1) * G4],
                            wiT_d[l][kc * 128:kc * 128 + ksz, :],
                        )
                    for kc in range(2):
                        nc.sync.dma_start(
                            whT_sb[l][:, kc * G4:(kc + 1) * G4],
                            whT_d[l][kc * 128:(kc + 1) * 128, :],
                        )
                    nc.sync.dma_start(bvec_sb[l][:], bvec_d[l][:])

            ones_sb = wpool.tile([1, VCHUNK], FP16, tag="ones", name="ones")
            nc.vector.memset(ones_sb[:], 1.0)
            ident = wpool.tile([128, 128], FP32, tag="ident", name="ident")
            make_identity(nc, ident[:])

            zrhs = wpool.tile([128, B_LOC], FP16, tag="zrhs", name="zrhs")
            nc.vector.memset(zrhs[:], 0.0)
            onesf = wpool.tile([1, VCHUNK], FP32, tag="onesf", name="onesf")
            nc.vector.memset(onesf[:], 1.0)
            # resident fcW^T: 2 K-chunks x [128, V] fp16 (64KB/partition each).
            # Tiles allocated here; the (large, ~45us) DMAs are emitted after
            # phase 1 so they stream during the LSTM instead of delaying it.
            fcw_sb = []
            for kc in range(2):
                fwt = wpool.tile([128, V], FP16, tag=f"fcw{kc}", name=f"fcw{kc}")
                fcw_sb.append(fwt)

            # persistent state
            xT = spool.tile([128, 2 * tb], FP16, tag="xT", name="xT")
            ht = [spool.tile([128, 2 * tb], FP16, tag=f"ht{l}", name=f"ht{l}")
                  for l in range(3)]
            ct = [spool.tile([128, 2 * Bb], FP32, tag=f"ct{l}", name=f"ct{l}")
                  for l in range(3)]
            for l in range(3):
                nc.vector.memset(ct[l][:], 0.0)

            # ---- Phase 1: embedding gather + transpose into xT ----
            with tc.tile_pool(name="embps", bufs=2, space="PSUM") as eps:
                for gt in range(n_gtiles):
                    p = min(128, ntok - gt * 128)
                    idt = work.tile([128, 1], mybir.dt.int32, tag="ids", name="ids")
                    nc.sync.dma_start(idt[0:p, :], xids_d[gt * 128:gt * 128 + p, :])
                    gat = work.tile([128, E], FP32, tag="gather", name="gather")
                    nc.gpsimd.indirect_dma_start(
                        out=gat[0:p, :],
                        out_offset=None,
                        in_=emb_d[:, :],
                        in_offset=bass.IndirectOffsetOnAxis(ap=idt[0:p, :1], axis=0),
                    )
                    for kc, ksz in enumerate(ksizes(E)):
                        tp = eps.tile([128, 128], FP32, tag="tpsum", name="tpsum")
                        nc.tensor.transpose(
                            tp[0:ksz, 0:p], gat[0:p, kc * 128:kc * 128 + ksz],
                            ident[0:p, 0:p],
                        )
                        nc.vector.tensor_copy(
                            xT[0:ksz, kc * tb + gt * 128:kc * tb + gt * 128 + p],
                            tp[0:ksz, 0:p],
                        )

            # LSTM weight loads after the gather chain, then the big fcW^T
            # loads last on the sync queue so they stream during the LSTM
            # phase (first consumer is the FC, hundreds of us later).
            emit_weight_dmas()
            for kc in range(2):
                nc.sync.dma_start(fcw_sb[kc][:], fcWT_d[kc * 128:(kc + 1) * 128, :])

            # ---- Phase 2: wavefront 3-layer LSTM ----
            # gate psum: 6 static slots (layer l, parity p) at col (2l+p)*SLOT,
            # each exactly one 2KB bank. Within a slot: m*cb + s*Bb.
            with tc.tile_pool(name="gatesps", bufs=1, space="PSUM") as gpp:
                gpt = [gpp.tile([128, SLOT], FP32, tag=f"gp{i}", name=f"gp{i}")
                       for i in range(6)]

                # The Tile scheduler orders each engine's queue from its own
                # cost-model simulation, which badly mis-times the tiny
                # recurrence matmuls; the resulting order serializes the three
                # layer chains. Pin every phase-2 instruction to its emission
                # order per engine (sync=False: pure ordering edges, no extra
                # semaphores) so the hand-built software pipeline is what the
                # hardware actually executes.
                _order_tail = {}

                def chain(engine, op):
                    prev = _order_tail.get(engine)
                    if prev is not None:
                        add_dep_helper(op.ins, prev, sync=False,
                                       reason="pin engine order")
                    _order_tail[engine] = op.ins
                    return op

                def emit_xg(l, cx):
                    ks = ksizes(LAYER_DIMS[l])
                    slot = l * 2 + (cx % 2)
                    opener = None
                    for m in range(N_MCHUNK):
                        for kc, ksz in enumerate(ks):
                            if l == 0:
                                rhs = xT[0:ksz,
                                         kc * tb + cx * chunk * Bb:
                                         kc * tb + (cx + 1) * chunk * Bb]
                            else:
                                base = kc * tb
                                rhs = ht[l - 1][0:ksz,
                                                base + cx * chunk * Bb:
                                                base + (cx + 1) * chunk * Bb]
                            is_open = m == 0 and kc == 0
                            mm = chain('pe', nc.tensor.matmul(
                                gpt[slot][:, m * cb:(m + 1) * cb],
                                lhsT=wiT_sb[l][0:ksz,
                                               kc * G4 + m * 128:kc * G4 + (m + 1) * 128],
                                rhs=rhs,
                                start=is_open,
                                stop=False,
                                skip_group_check=True,
                            ))
                            if is_open:
                                opener = mm.ins
                            elif kc == 0:
                                add_dep_helper(mm.ins, opener, sync=False,
                                               reason="slot opener order")
                        chain('pe', nc.tensor.matmul(
                            gpt[slot][:, m * cb:(m + 1) * cb],
                            lhsT=bvec_sb[l][:, m * 128:(m + 1) * 128],
                            rhs=ones_sb[:, 0:cb],
                            start=False,
                            stop=False,
                            skip_group_check=True,
                        ))

                n_blocks = nch + 2

                # Per-(wave, layer) "slot" emission, software-pipelined: a
                # slot emits its own matmuls + sigmoid/tanh(g) + c-update;
                # its tanh(c) + h-write are emitted one SLOT later (after the
                # next layer's slot in the same cycle, or at end of cycle for
                # the last layer). This keeps the h-write's engine-queue
                # position early enough that the next cycle's matmuls for the
                # same layer never wait a full extra cycle, while avoiding
                # head-of-line blocking of the following slot's activations.
                pending = []

                def emit_tc_h(l, w):
                    tl = w - lag * l
                    tct = work.tile([128, 2 * Bb], FP32, tag="tct", name="tct",
                                    bufs=4)
                    chain('act', nc.scalar.activation(tct[:], ct[l][:], AF.Tanh))
                    chain('dve', nc.vector.tensor_mul(
                        mkap(ht[l][:], tl * Bb, [[tb, 2], [1, Bb]]),
                        mkap(osave[l][:], 4 * Bb, [[Bb, 2], [1, Bb]]),
                        mkap(tct[:], 0, [[Bb, 2], [1, Bb]]),
                    ))

                def flush_pending(only_l=None):
                    for j in range(len(pending) - 1, -1, -1):
                        l, w = pending[j]
                        if only_l is None or l == only_l:
                            emit_tc_h(l, w)
                            pending.pop(j)

                osave = [None, None, None]

                def emit_slot(l, w):
                    s = w % chunk
                    wb = w // chunk
                    tl = w - lag * l
                    slot = l * 2 + (wb - l) % 2
                    # 16 recurrent matmuls accumulate onto xg in the live slot
                    for kc in range(2):
                        if tl == 0:
                            rhs = zrhs[:, 0:Bb]
                        else:
                            rhs = ht[l][:, kc * tb + (tl - 1) * Bb:
                                        kc * tb + tl * Bb]
                        for m in range(N_MCHUNK):
                            chain('pe', nc.tensor.matmul(
                                gpt[slot][:, m * cb + s * Bb:
                                          m * cb + (s + 1) * Bb],
                                lhsT=whT_sb[l][:, kc * G4 + m * 128:
                                               kc * G4 + (m + 1) * 128],
                                rhs=rhs,
                                start=False,
                                stop=(kc == 1),
                                skip_group_check=True,
                            ))
                    gb = s * Bb
                    sig = work.tile([128, 6 * Bb], FP32, tag="sig", name="sig",
                                    bufs=4)
                    gg = work.tile([128, 2 * Bb], FP32, tag="gg", name="gg",
                                   bufs=4)
                    t1t = work.tile([128, 2 * Bb], FP32, tag="t1t", name="t1t",
                                    bufs=4)
                    t2t = work.tile([128, 2 * Bb], FP32, tag="t2t", name="t2t",
                                    bufs=4)
                    chain('act', nc.scalar.activation(
                        mkap(sig[:], 0, [[Bb, 6], [1, Bb]]),
                        mkap(gpt[slot][:], gb, [[cb, 6], [1, Bb]]),
                        AF.Sigmoid,
                    ))
                    chain('act', nc.scalar.activation(
                        mkap(gg[:], 0, [[Bb, 2], [1, Bb]]),
                        mkap(gpt[slot][:], gb + 6 * cb, [[cb, 2], [1, Bb]]),
                        AF.Tanh,
                    ))
                    chain('dve', nc.vector.tensor_mul(
                        t1t[:], sig[:, 2 * Bb:4 * Bb], ct[l][:]))
                    chain('dve', nc.vector.tensor_mul(
                        t2t[:], sig[:, 0:2 * Bb], gg[:]))
                    chain('dve', nc.vector.tensor_add(ct[l][:], t1t[:], t2t[:]))
                    osave[l] = sig  # o-gate slice read by the delayed h-write
                    pending.append((l, w))

                for wb in range(n_blocks):
                    for s in range(chunk):
                        w = wb * chunk + s
                        active = [l for l in range(3) if l <= wb < l + nch]
                        for l in active:
                            if s == 0:
                                # this block's xg for layer l, interleaved
                                # right before the layer's first slot so the
                                # other layers' blocks aren't delayed by one
                                # big xg burst (inputs all flushed last cycle)
                                emit_xg(l, wb - l)
                            flush_pending(only_l=l)  # warmup edge safety net
                            emit_slot(l, w)
                            # previous layer's h-write, one slot of slack
                            for j in range(len(pending) - 1, -1, -1):
                                lp, wp = pending[j]
                                if lp == l - 1 and wp == w:
                                    emit_tc_h(lp, wp)
                                    pending.pop(j)
                        flush_pending()  # last active layer at end of cycle

            # ---- Phase 3: FC + log_softmax ----
            # Pass 1 per m-chunk: logits -> exp (ACT, 4-bank-wide ops) with
            # per-row accumulator sums; then lse. Pass 2: recompute logits,
            # subtract lse, DMA out (bf16). Pass 1 of m+1 and pass 2 of m are
            # interleaved at CHUNK granularity so the two consumer streams
            # (exp on ACT, subtract on DVE) both stay fed and the phase runs
            # at the PE matmul rate. The final m's pass-2 subtracts alternate
            # DVE / ACT(Identity + per-token bias) since no exps run then.
            with tc.tile_pool(name="fcps", bufs=4, space="PSUM") as fcpp:
                VP = 2 * VCHUNK  # 1024: one psum tile spans 2 banks
                n_m = (tb + 127) // 128
                n_vp = (V + VP - 1) // VP
                zacc = spool.tile([128, n_m * n_vp], FP32, tag="zacc", name="zacc")
                neglse = spool.tile([128, n_m], FP32, tag="neglse", name="neglse")

                def fc_mms(m, vp, msz):
                    ps = fcpp.tile([128, VP], FP32, tag="fcpsum", name="fcpsum")
                    vbase = vp * VP
                    for sub in range(2):
                        vs = vbase + sub * VCHUNK
                        if vs >= V:
                            continue
                        vsz = min(VCHUNK, V - vs)
                        for kc in range(2):
                            chain('pe', nc.tensor.matmul(
                                ps[0:msz, sub * VCHUNK:sub * VCHUNK + vsz],
                                lhsT=ht[2][:, kc * tb + m * 128:
                                           kc * tb + m * 128 + msz],
                                rhs=fcw_sb[kc][:, vs:vs + vsz],
                                start=(kc == 0),
                                stop=(kc == 1),
                                skip_group_check=True,
                            ))
                        if has_bias:
                            fb = fcwpool.tile([1, VCHUNK], FP32, tag="fcb", name="fcb")
                            nc.sync.dma_start(fb[:, 0:vsz], fcb_d[:, vs:vs + vsz])
                            chain('pe', nc.tensor.matmul(
                                ps[0:msz, sub * VCHUNK:sub * VCHUNK + vsz],
                                lhsT=onesf[:, 0:msz],
                                rhs=fb[:, 0:vsz],
                                start=False,
                                stop=True,
                                skip_group_check=True,
                            ))
                    return ps

                def p1_chunk(m, vp):
                    msz = min(128, tb - m * 128)
                    vpsz = min(VP, V - vp * VP)
                    ps = fc_mms(m, vp, msz)
                    esc = stpool.tile([128, VP], FP32, tag="expsc", name="expsc")
                    chain('act', nc.scalar.activation(
                        esc[0:msz, 0:vpsz], ps[0:msz, 0:vpsz], AF.Exp,
                        accum_out=zacc[0:msz, m * n_vp + vp:m * n_vp + vp + 1],
                    ))

                def p1_finish(m):
                    msz = min(128, tb - m * 128)
                    zs = work.tile([128, 1], FP32, tag="zsum", name="zsum")
                    chain('dve', nc.vector.tensor_reduce(
                        zs[0:msz], zacc[0:msz, m * n_vp:(m + 1) * n_vp],
                        op=mybir.AluOpType.add, axis=mybir.AxisListType.X,
                    ))
                    lse = work.tile([128, 1], FP32, tag="lse", name="lse")
                    chain('act', nc.scalar.activation(lse[0:msz], zs[0:msz], AF.Ln))
                    chain('dve', nc.vector.tensor_scalar_mul(
                        neglse[0:msz, m:m + 1], lse[0:msz], -1.0))

                def p2_chunk(m, vp, on_act=False):
                    msz = min(128, tb - m * 128)
                    vpsz = min(VP, V - vp * VP)
                    ps = fc_mms(m, vp, msz)
                    st = stpool.tile([128, VP], mybir.dt.bfloat16, tag="stage",
                                     name="stage")
                    if on_act:
                        # no exps run in the final-m tail, so half the
                        # subtracts ride the ACT engine (Identity + bias)
                        chain('act', nc.scalar.activation(
                            st[0:msz, 0:vpsz], ps[0:msz, 0:vpsz], AF.Identity,
                            bias=neglse[0:msz, m:m + 1],
                        ))
                    else:
                        chain('dve', nc.vector.tensor_scalar_add(
                            st[0:msz, 0:vpsz], ps[0:msz, 0:vpsz],
                            neglse[0:msz, m:m + 1],
                        ))
                    nc.sync.dma_start(
                        out_d[m * 128:m * 128 + msz, vp * VP:vp * VP + vpsz],
                        st[0:msz, 0:vpsz],
                    )

                for vp in range(n_vp):
                    p1_chunk(0, vp)
                p1_finish(0)
                for m in range(1, n_m):
                    for vp in range(n_vp):
                        p1_chunk(m, vp)
                        p2_chunk(m - 1, vp)
                    p1_finish(m)
                for vp in range(n_vp):
                    p2_chunk(n_m - 1, vp, on_act=(vp % 2 == 0))

    nc.compile()
    return nc


_nc_cache = {}


def _get_nc(t_steps, has_bias=False):
    key = (t_steps, has_bias)
    if key not in _nc_cache:
        _nc_cache[key] = build_nc(t_steps, has_bias)
    return _nc_cache[key]


def prep_inputs(x, emb, Wi, Wh, bb, fcW, fcb, t_steps=T):
    """Host-side shard + repack. Returns in_maps for the 8 cores."""
    perm = np.concatenate([np.arange(0, 512), np.arange(768, 1024),
                           np.arange(512, 768)])  # i,f | o | g
    shared = {
        "emb": np.ascontiguousarray(emb.astype(np.float32)),
        "fcWT": np.ascontiguousarray(fcW.T.astype(np.float16)),
        "fcb": np.ascontiguousarray(fcb[None, :].astype(np.float16)),
    }
    for l in range(3):
        shared[f"wiT{l}"] = np.ascontiguousarray(Wi[l][perm].T.astype(np.float16))
        shared[f"whT{l}"] = np.ascontiguousarray(Wh[l][perm].T.astype(np.float16))
        shared[f"bvec{l}"] = np.ascontiguousarray(bb[l][perm][None, :].astype(np.float16))
    in_maps = []
    for c in range(N_CORES):
        x_loc = x[c * B_LOC:(c + 1) * B_LOC, :t_steps]  # [B_LOC, t]
        xids = np.ascontiguousarray(
            x_loc.T.reshape(-1, 1).astype(np.int32))  # [(t b), 1]
        m = dict(shared)
        m["xids"] = xids
        in_maps.append(m)
    return in_maps


def kernel(x, emb, Wi0, Wh0, b0, Wi1, Wh1, b1, Wi2, Wh2, b2, fcW, fcb,
           t_steps=T, trace=False):
    x = np.asarray(x)
    has_bias = bool(np.any(np.asarray(fcb)))
    nc = _get_nc(t_steps, has_bias)
    in_maps = prep_inputs(
        np.asarray(x), np.asarray(emb),
        [np.asarray(Wi0), np.asarray(Wi1), np.asarray(Wi2)],
        [np.asarray(Wh0), np.asarray(Wh1), np.asarray(Wh2)],
        [np.asarray(b0), np.asarray(b1), np.asarray(b2)],
        np.asarray(fcW), np.asarray(fcb), t_steps)
    res = run_bass_kernel_spmd(nc, in_maps, core_ids=list(range(N_CORES)),
                               trace=trace)
    out = np.empty((B, t_steps, V), np.float32)
    for c in range(N_CORES):
        oc = np.asarray(res.results[c]["out"]).astype(np.float32)
        oc = oc.reshape(t_steps, B_LOC, V)
        out[c * B_LOC:(c + 1) * B_LOC] = oc.transpose(1, 0, 2)
    kernel.last_results = res
    return out



# revision 38
# speedup vs baseline: 1.0387x; 1.0387x over previous
"""Trainium2 Bass kernel: 3-layer LSTM LM (embed -> 3xLSTM(H=256) -> FC 32000 -> log_softmax).

Strategy: data-parallel over batch across 8 cores (2 sequences per core).
Everything else (LSTM recurrence, FC, log_softmax over full vocab) is local
per core; zero collectives.
"""

import sys

sys.path.insert(0, "/opt/trn_rl_repo")

import numpy as np

import concourse.bass as bass
import concourse.mybir as mybir
import concourse.tile as tile
from concourse import bacc
from concourse.bass_utils import run_bass_kernel_spmd
from concourse.masks import make_identity
from concourse.tile import add_dep_helper

# Problem dims
V = 32000
E = 200
H = 256
B = 16
T = 256
N_CORES = 8
B_LOC = B // N_CORES  # 2 sequences per core
G4 = 4 * H  # 1024 gate width

# Tiling
CHUNK = 16  # recurrence steps per xg-precompute chunk (also the layer lag)
N_MCHUNK = G4 // 128  # 8 gate row chunks
VCHUNK = 512
SLOT = 512  # gate psum slot stride in fp32 elems == one 2KB bank
FP16 = mybir.dt.float16
FP32 = mybir.dt.float32
AF = mybir.ActivationFunctionType
LAYER_DIMS = [E, H, H]


def mkap(tile_ap, off, dims):
    """Custom strided AP on a tile: off in elements, dims=[[step,count],...]."""
    return bass.AP(tile_ap.tensor, off,
                   [list(tile_ap.ap[0])] + [list(d) for d in dims])


def ksizes(dim):
    """Partition-chunk sizes for a contraction dim."""
    out = []
    while dim > 0:
        out.append(min(dim, 128))
        dim -= 128
    return out


def build_nc(t_steps=T, has_bias=False):
    nsteps = t_steps
    nchunks = nsteps // CHUNK if nsteps >= CHUNK else 1
    chunk = min(CHUNK, nsteps)
    cb = chunk * B_LOC
    tb = nsteps * B_LOC
    ntok = nsteps * B_LOC
    n_gtiles = (ntok + 127) // 128

    nc = bacc.Bacc("TRN2", target_bir_lowering=False, debug=False,
                   num_devices=N_CORES)

    # DRAM I/O
    xids_d = nc.dram_tensor("xids", [ntok, 1], mybir.dt.int32, kind="ExternalInput")
    emb_d = nc.dram_tensor("emb", [V, E], FP32, kind="ExternalInput")
    wiT_d = [nc.dram_tensor(f"wiT{l}", [LAYER_DIMS[l], G4], FP16, kind="ExternalInput")
             for l in range(3)]
    whT_d = [nc.dram_tensor(f"whT{l}", [H, G4], FP16, kind="ExternalInput")
             for l in range(3)]
    bvec_d = [nc.dram_tensor(f"bvec{l}", [1, G4], FP16, kind="ExternalInput")
              for l in range(3)]
    fcWT_d = nc.dram_tensor("fcWT", [H, V], FP16, kind="ExternalInput")
    fcb_d = nc.dram_tensor("fcb", [1, V], FP16, kind="ExternalInput")
    # bf16 output (halves the output DMA); host casts back to fp32
    out_d = nc.dram_tensor("out", [tb, V], mybir.dt.bfloat16, kind="ExternalOutput")

    with tile.TileContext(nc, num_cores=N_CORES) as tc:
        with (
            tc.tile_pool(name="weights", bufs=1) as wpool,
            tc.tile_pool(name="state", bufs=1) as spool,
            tc.tile_pool(name="work", bufs=3) as work,
            tc.tile_pool(name="fcw", bufs=3) as fcwpool,
            tc.tile_pool(name="stage", bufs=3) as stpool,
        ):
            Bb = B_LOC
            lag = chunk
            nch = max(1, nsteps // chunk)

            # ---- Phase 0: allocate weight tiles (DMAs emitted after the
            # embedding gather so the gather-transpose chain starts first) ----
            wiT_sb = []
            whT_sb = []
            bvec_sb = []
            for l in range(3):
                ks = ksizes(LAYER_DIMS[l])
                wi = wpool.tile([128, len(ks) * G4], FP16, tag=f"wiT{l}", name=f"wiT{l}")
                wiT_sb.append(wi)
                wh = wpool.tile([128, 2 * G4], FP16, tag=f"whT{l}", name=f"whT{l}")
                whT_sb.append(wh)
                bv = wpool.tile([1, G4], FP16, tag=f"bvec{l}", name=f"bvec{l}")
                bvec_sb.append(bv)

            def emit_weight_dmas():
                for l in range(3):
                    ks = ksizes(LAYER_DIMS[l])
                    for kc, ksz in enumerate(ks):
                        nc.sync.dma_start(
                            wiT_sb[l][0:ksz, kc * G4:(kc + 1) * G4],
                            wiT_d[l][kc * 128:kc * 128 + ksz, :],
                        )
                    for kc in range(2):
                        nc.sync.dma_start(
                            whT_sb[l][:, kc * G4:(kc + 1) * G4],
                            whT_d[l][kc * 128:(kc + 1) * 128, :],
                        )
                    nc.sync.dma_start(bvec_sb[l][:], bvec_d[l][:])

            ones_sb = wpool.tile([1, VCHUNK], FP16, tag="ones", name="ones")
            nc.vector.memset(ones_sb[:], 1.0)
            ident = wpool.tile([128, 128], FP32, tag="ident", name="ident")
            make_identity(nc, ident[:])

            zrhs = wpool.tile([128, B_LOC], FP16, tag="zrhs", name="zrhs")
            nc.vector.memset(zrhs[:], 0.0)
            onesf = wpool.tile([1, VCHUNK], FP32, tag="onesf", name="onesf")
            nc.vector.memset(onesf[:], 1.0)
            # resident fcW^T: 2 K-chunks x [128, V] fp16 (64KB/partition each).
            # Tiles allocated here; the (large, ~45us) DMAs are emitted after
            # phase 1 so they stream during the LSTM instead of delaying it.
            fcw_sb = []
            for kc in range(2):
                fwt = wpool.tile([128, V], FP16, tag=f"fcw{kc}", name=f"fcw{kc}")
                fcw_sb.append(fwt)

            # persistent state
            xT = spool.tile([128, 2 * tb], FP16, tag="xT", name="xT")
            ht = [spool.tile([128, 2 * tb], FP16, tag=f"ht{l}", name=f"ht{l}")
                  for l in range(3)]
            ct = [spool.tile([128, 2 * Bb], FP32, tag=f"ct{l}", name=f"ct{l}")
                  for l in range(3)]
            for l in range(3):
                nc.vector.memset(ct[l][:], 0.0)

            # ---- Phase 1: embedding gather + transpose into xT ----
            with tc.tile_pool(name="embps", bufs=2, space="PSUM") as eps:
                for gt in range(n_gtiles):
                    p = min(128, ntok - gt * 128)
                    idt = work.tile([128, 1], mybir.dt.int32, tag="ids", name="ids")
                    nc.sync.dma_start(idt[0:p, :], xids_d[gt * 128:gt * 128 + p, :])
                    gat = work.tile([128, E], FP32, tag="gather", name="gather")
                    nc.gpsimd.indirect_dma_start(
                        out=gat[0:p, :],
                        out_offset=None,
                        in_=emb_d[:, :],
                        in_offset=bass.IndirectOffsetOnAxis(ap=idt[0:p, :1], axis=0),
                    )
                    for kc, ksz in enumerate(ksizes(E)):
                        tp = eps.tile([128, 128], FP32, tag="tpsum", name="tpsum")
                        nc.tensor.transpose(
                            tp[0:ksz, 0:p], gat[0:p, kc * 128:kc * 128 + ksz],
                            ident[0:p, 0:p],
                        )
                        nc.vector.tensor_copy(
                            xT[0:ksz, kc * tb + gt * 128:kc * tb + gt * 128 + p],
                            tp[0:ksz, 0:p],
                        )

            # LSTM weight loads after the gather chain, then the big fcW^T
            # loads last on the sync queue so they stream during the LSTM
            # phase (first consumer is the FC, hundreds of us later).
            emit_weight_dmas()
            for kc in range(2):
                nc.sync.dma_start(fcw_sb[kc][:], fcWT_d[kc * 128:(kc + 1) * 128, :])

            # ---- Phase 2: wavefront 3-layer LSTM ----
            # gate psum: 6 static slots (layer l, parity p) at col (2l+p)*SLOT,
            # each exactly one 2KB bank. Within a slot: m*cb + s*Bb.
            with tc.tile_pool(name="gatesps", bufs=1, space="PSUM") as gpp:
                gpt = [gpp.tile([128, SLOT], FP32, tag=f"gp{i}", name=f"gp{i}")
                       for i in range(6)]

                # The Tile scheduler orders each engine's queue from its own
                # cost-model simulation, which badly mis-times the tiny
                # recurrence matmuls; the resulting order serializes the three
                # layer chains. Pin every phase-2 instruction to its emission
                # order per engine (sync=False: pure ordering edges, no extra
                # semaphores) so the hand-built software pipeline is what the
                # hardware actually executes.
                _order_tail = {}

                def chain(engine, op):
                    prev = _order_tail.get(engine)
                    if prev is not None:
                        add_dep_helper(op.ins, prev, sync=False,
                                       reason="pin engine order")
                    _order_tail[engine] = op.ins
                    return op

                def emit_xg(l, cx):
                    ks = ksizes(LAYER_DIMS[l])
                    slot = l * 2 + (cx % 2)
                    opener = None
                    for m in range(N_MCHUNK):
                        for kc, ksz in enumerate(ks):
                            if l == 0:
                                rhs = xT[0:ksz,
                                         kc * tb + cx * chunk * Bb:
                                         kc * tb + (cx + 1) * chunk * Bb]
                            else:
                                base = kc * tb
                                rhs = ht[l - 1][0:ksz,
                                                base + cx * chunk * Bb:
                                                base + (cx + 1) * chunk * Bb]
                            is_open = m == 0 and kc == 0
                            mm = chain('pe', nc.tensor.matmul(
                                gpt[slot][:, m * cb:(m + 1) * cb],
                                lhsT=wiT_sb[l][0:ksz,
                                               kc * G4 + m * 128:kc * G4 + (m + 1) * 128],
                                rhs=rhs,
                                start=is_open,
                                stop=False,
                                skip_group_check=True,
                            ))
                            if is_open:
                                opener = mm.ins
                            elif kc == 0:
                                add_dep_helper(mm.ins, opener, sync=False,
                                               reason="slot opener order")
                        chain('pe', nc.tensor.matmul(
                            gpt[slot][:, m * cb:(m + 1) * cb],
                            lhsT=bvec_sb[l][:, m * 128:(m + 1) * 128],
                            rhs=ones_sb[:, 0:cb],
                            start=False,
                            stop=False,
                            skip_group_check=True,
                        ))

                n_blocks = nch + 2

                # Per-(wave, layer) "slot" emission, software-pipelined: a
                # slot emits its own matmuls + sigmoid/tanh(g) + c-update;
                # its tanh(c) + h-write are emitted one SLOT later (after the
                # next layer's slot in the same cycle, or at end of cycle for
                # the last layer). This keeps the h-write's engine-queue
                # position early enough that the next cycle's matmuls for the
                # same layer never wait a full extra cycle, while avoiding
                # head-of-line blocking of the following slot's activations.
                pending = []

                def emit_tc_h(l, w):
                    tl = w - lag * l
                    tct = work.tile([128, 2 * Bb], FP32, tag="tct", name="tct",
                                    bufs=4)
                    chain('act', nc.scalar.activation(tct[:], ct[l][:], AF.Tanh))
                    chain('dve', nc.vector.tensor_mul(
                        mkap(ht[l][:], tl * Bb, [[tb, 2], [1, Bb]]),
                        mkap(osave[l][:], 4 * Bb, [[Bb, 2], [1, Bb]]),
                        mkap(tct[:], 0, [[Bb, 2], [1, Bb]]),
                    ))

                def flush_pending(only_l=None):
                    for j in range(len(pending) - 1, -1, -1):
                        l, w = pending[j]
                        if only_l is None or l == only_l:
                            emit_tc_h(l, w)
                            pending.pop(j)

                osave = [None, None, None]

                def emit_slot(l, w):
                    s = w % chunk
                    wb = w // chunk
                    tl = w - lag * l
                    slot = l * 2 + (wb - l) % 2
                    # 16 recurrent matmuls accumulate onto xg in the live slot
                    for kc in range(2):
                        if tl == 0:
                            rhs = zrhs[:, 0:Bb]
                        else:
                            rhs = ht[l][:, kc * tb + (tl - 1) * Bb:
                                        kc * tb + tl * Bb]
                        for m in range(N_MCHUNK):
                            chain('pe', nc.tensor.matmul(
                                gpt[slot][:, m * cb + s * Bb:
                                          m * cb + (s + 1) * Bb],
                                lhsT=whT_sb[l][:, kc * G4 + m * 128:
                                               kc * G4 + (m + 1) * 128],
                                rhs=rhs,
                                start=False,
                                stop=(kc == 1),
                                skip_group_check=True,
                            ))
                    gb = s * Bb
                    sig = work.tile([128, 6 * Bb], FP32, tag="sig", name="sig",
                                    bufs=4)
                    gg = work.tile([128, 2 * Bb], FP32, tag="gg", name="gg",
                                   bufs=4)
                    t1t = work.tile([128, 2 * Bb], FP32, tag="t1t", name="t1t",
                                    bufs=4)
                    t2t = work.tile([128, 2 * Bb], FP32, tag="t2t", name="t2t",
                                    bufs=4)
                    chain('act', nc.scalar.activation(
                        mkap(sig[:], 0, [[Bb, 6], [1, Bb]]),
                        mkap(gpt[slot][:], gb, [[cb, 6], [1, Bb]]),
                        AF.Sigmoid,
                    ))
                    chain('act', nc.scalar.activation(
                        mkap(gg[:], 0, [[Bb, 2], [1, Bb]]),
                        mkap(gpt[slot][:], gb + 6 * cb, [[cb, 2], [1, Bb]]),
                        AF.Tanh,
                    ))
                    chain('dve', nc.vector.tensor_mul(
                        t1t[:], sig[:, 2 * Bb:4 * Bb], ct[l][:]))
                    chain('dve', nc.vector.tensor_mul(
                        t2t[:], sig[:, 0:2 * Bb], gg[:]))
                    chain('dve', nc.vector.tensor_add(ct[l][:], t1t[:], t2t[:]))
                    osave[l] = sig  # o-gate slice read by the delayed h-write
                    pending.append((l, w))

                for wb in range(n_blocks):
                    for s in range(chunk):
                        w = wb * chunk + s
                        active = [l for l in range(3) if l <= wb < l + nch]
                        for l in active:
                            if s == 0:
                                # this block's xg for layer l, interleaved
                                # right before the layer's first slot so the
                                # other layers' blocks aren't delayed by one
                                # big xg burst (inputs all flushed last cycle)
                                emit_xg(l, wb - l)
                            flush_pending(only_l=l)  # warmup edge safety net
                            emit_slot(l, w)
                            # previous layer's h-write, one slot of slack
                            for j in range(len(pending) - 1, -1, -1):
                                lp, wp = pending[j]
                                if lp == l - 1 and wp == w:
                                    emit_tc_h(lp, wp)
                                    pending.pop(j)
                        flush_pending()  # last active layer at end of cycle

            # ---- Phase 3: FC + log_softmax ----
            # Pass 1 per m-chunk: logits -> exp (ACT, 4-bank-wide ops) with
            # per-row accumulator sums; then lse. Pass 2: recompute logits,
            # subtract lse, DMA out (bf16). Pass 1 of m+1 and pass 2 of m are
            # interleaved at CHUNK granularity so the two consumer streams
            # (exp on ACT, subtract on DVE) both stay fed and the phase runs
            # at the PE matmul rate. The final m's pass-2 subtracts alternate
            # DVE / ACT(Identity + per-token bias) since no exps run then.
            with tc.tile_pool(name="fcps", bufs=4, space="PSUM") as fcpp:
                VP = 2 * VCHUNK  # 1024: one psum tile spans 2 banks
                n_m = (tb + 127) // 128
                n_vp = (V + VP - 1) // VP
                zacc = spool.tile([128, n_m * n_vp], FP32, tag="zacc", name="zacc")
                neglse = spool.tile([128, n_m], FP32, tag="neglse", name="neglse")

                def fc_mms(m, vp, msz):
                    ps = fcpp.tile([128, VP], FP32, tag="fcpsum", name="fcpsum")
                    vbase = vp * VP
                    for sub in range(2):
                        vs = vbase + sub * VCHUNK
                        if vs >= V:
                            continue
                        vsz = min(VCHUNK, V - vs)
                        for kc in range(2):
                            chain('pe', nc.tensor.matmul(
                                ps[0:msz, sub * VCHUNK:sub * VCHUNK + vsz],
                                lhsT=ht[2][:, kc * tb + m * 128:
                                           kc * tb + m * 128 + msz],
                                rhs=fcw_sb[kc][:, vs:vs + vsz],
                                start=(kc == 0),
                                stop=(kc == 1),
                                skip_group_check=True,
                            ))
                        if has_bias:
                            fb = fcwpool.tile([1, VCHUNK], FP32, tag="fcb", name="fcb")
                            nc.sync.dma_start(fb[:, 0:vsz], fcb_d[:, vs:vs + vsz])
                            chain('pe', nc.tensor.matmul(
                                ps[0:msz, sub * VCHUNK:sub * VCHUNK + vsz],
                                lhsT=onesf[:, 0:msz],
                                rhs=fb[:, 0:vsz],
                                start=False,
                                stop=True,
                                skip_group_check=True,
                            ))
                    return ps

                def p1_chunk(m, vp):
                    msz = min(128, tb - m * 128)
                    vpsz = min(VP, V - vp * VP)
                    ps = fc_mms(m, vp, msz)
                    esc = stpool.tile([128, VP], FP32, tag="expsc", name="expsc")
                    chain('act', nc.scalar.activation(
                        esc[0:msz, 0:vpsz], ps[0:msz, 0:vpsz], AF.Exp,
                        accum_out=zacc[0:msz, m * n_vp + vp:m * n_vp + vp + 1],
                    ))

                def p1_finish(m):
                    msz = min(128, tb - m * 128)
                    zs = work.tile([128, 1], FP32, tag="zsum", name="zsum")
                    chain('dve', nc.vector.tensor_reduce(
                        zs[0:msz], zacc[0:msz, m * n_vp:(m + 1) * n_vp],
                        op=mybir.AluOpType.add, axis=mybir.AxisListType.X,
                    ))
                    lse = work.tile([128, 1], FP32, tag="lse", name="lse")
                    chain('act', nc.scalar.activation(lse[0:msz], zs[0:msz], AF.Ln))
                    chain('dve', nc.vector.tensor_scalar_mul(
                        neglse[0:msz, m:m + 1], lse[0:msz], -1.0))

                def p2_chunk(m, vp, on_act=False):
                    msz = min(128, tb - m * 128)
                    vpsz = min(VP, V - vp * VP)
                    ps = fc_mms(m, vp, msz)
                    st = stpool.tile([128, VP], mybir.dt.bfloat16, tag="stage",
                                     name="stage")
                    if on_act:
                        # no exps run in the final-m tail, so half the
                        # subtracts ride the ACT engine (Identity + bias)
                        chain('act', nc.scalar.activation(
                            st[0:msz, 0:vpsz], ps[0:msz, 0:vpsz], AF.Identity,
                            bias=neglse[0:msz, m:m + 1],
                        ))
                    else:
                        chain('dve', nc.vector.tensor_scalar_add(
                            st[0:msz, 0:vpsz], ps[0:msz, 0:vpsz],
                            neglse[0:msz, m:m + 1],
                        ))
                    nc.sync.dma_start(
                        out_d[m * 128:m * 128 + msz, vp * VP:vp * VP + vpsz],
                        st[0:msz, 0:vpsz],
                    )

                for vp in range(n_vp):
                    p1_chunk(0, vp)
                p1_finish(0)
                for m in range(1, n_m):
                    for vp in range(n_vp):
                        p1_chunk(m, vp)
                        p2_chunk(m - 1, vp)
                    p1_finish(m)
                for vp in range(n_vp):
                    p2_chunk(n_m - 1, vp, on_act=(vp % 2 == 0))

    nc.compile()
    return nc


_nc_cache = {}


def _get_nc(t_steps, has_bias=False):
    key = (t_steps, has_bias)
    if key not in _nc_cache:
        _nc_cache[key] = build_nc(t_steps, has_bias)
    return _nc_cache[key]


def prep_inputs(x, emb, Wi, Wh, bb, fcW, fcb, t_steps=T):
    """Host-side shard + repack. Returns in_maps for the 8 cores."""
    perm = np.concatenate([np.arange(0, 512), np.arange(768, 1024),
                           np.arange(512, 768)])  # i,f | o | g
    shared = {
        "emb": np.ascontiguousarray(emb.astype(np.float32)),
        "fcWT": np.ascontiguousarray(fcW.T.astype(np.float16)),
        "fcb": np.ascontiguousarray(fcb[None, :].astype(np.float16)),
    }
    for l in range(3):
        shared[f"wiT{l}"] = np.ascontiguousarray(Wi[l][perm].T.astype(np.float16))
        shared[f"whT{l}"] = np.ascontiguousarray(Wh[l][perm].T.astype(np.float16))
        shared[f"bvec{l}"] = np.ascontiguousarray(bb[l][perm][None, :].astype(np.float16))
    in_maps = []
    for c in range(N_CORES):
        x_loc = x[c * B_LOC:(c + 1) * B_LOC, :t_steps]  # [B_LOC, t]
        xids = np.ascontiguousarray(
            x_loc.T.reshape(-1, 1).astype(np.int32))  # [(t b), 1]
        m = dict(shared)
        m["xids"] = xids
        in_maps.append(m)
    return in_maps


def kernel(x, emb, Wi0, Wh0, b0, Wi1, Wh1, b1, Wi2, Wh2, b2, fcW, fcb,
           t_steps=T, trace=False):
    x = np.asarray(x)
    has_bias = bool(np.any(np.asarray(fcb)))
    nc = _get_nc(t_steps, has_bias)
    in_maps = prep_inputs(
        np.asarray(x), np.asarray(emb),
        [np.asarray(Wi0), np.asarray(Wi1), np.asarray(Wi2)],
        [np.asarray(Wh0), np.asarray(Wh1), np.asarray(Wh2)],
        [np.asarray(b0), np.asarray(b1), np.asarray(b2)],
        np.asarray(fcW), np.asarray(fcb), t_steps)
    res = run_bass_kernel_spmd(nc, in_maps, core_ids=list(range(N_CORES)),
                               trace=trace)
    out = np.empty((B, t_steps, V), np.float32)
    for c in range(N_CORES):
        oc = np.asarray(res.results[c]["out"]).astype(np.float32)
        oc = oc.reshape(t_steps, B_LOC, V)
        out[c * B_LOC:(c + 1) * B_LOC] = oc.transpose(1, 0, 2)
    kernel.last_results = res
    return out

